# revision 28
# baseline (speedup 1.0000x reference)
"""Trainium2 Bass kernel for causal ("FORWARD" direction) multi-head attention.

Reference computation (per batch b, n_heads=8, d=128):
  Q = x @ Wq.T ; K = x @ Wk.T ; V = x @ Wv.T          (nn.Linear, no bias)
  scores[h,i,j] = (Qh[i] . Kh[j]) / sqrt(d)
  scores += -10000 where j <= i   (keeps strict upper triangle j > i)
  attn = softmax(scores, axis=j) ; out = attn @ Vh ; concat heads
  Row i=1023 is fully masked; jax softmax's max-subtraction makes it equal
  softmax of the *raw* scores, so the kernel keeps column i=1023 unmasked.

Sharding: data-parallel over batch B=8 -> 8 cores, no collectives.

Device layout (per core, everything transposed so the softmax reduction is a
matmul-friendly partition-dim reduction):
  xT[k,t]       : x.T                                  [1024,1024]
  qT/kT[o,t]    : per head-group of 4 heads            via Wq.T/Wk.T as lhsT
  v[t,o]        : natural V                            via xT as lhsT
  S_T[j,i]      = kT_tile.T @ qT  (contraction over d=128, single tile)
  expS          = exp(S_T + adder)   (adder patterns precomputed on host)
  U_T[dd,i]     = sum_j V[j,dd] expS[j,i]   (matmul accum over j tiles)
  colsum[*,i]   = ones.T @ expS             (partition-broadcast row of sums)
  out_T         = U_T * reciprocal(colsum) -> DRAM (host transposes back)

Scheduling: projections for the NEXT head-group are emitted interleaved with
attention of the current group, so PE fills the gaps where it would otherwise
wait on Activation (exp) results.
"""

import os
import sys
from collections import deque

import numpy as np

if "/opt/trn_rl_repo" not in sys.path:
    sys.path.insert(0, "/opt/trn_rl_repo")

B, T, D, H, DH = 8, 1024, 1024, 8, 128
P = 128          # partition tile
NI = 512         # i-chunk (moving free size)
NG, GH = 2, 4    # head groups x heads per group
NKT = T // P     # 8 contraction tiles
MM_DT = os.environ.get("KERNEL_MM_DT", "bf16")  # f32r | f32 | bf16

# experiment knobs (production defaults; probes override via module attr)
KOPT = {
    "acc_eng": "pool",  # eacc chain engine: dve | pool | alt(ernate by unit)
    "adders": True,      # apply mask adders (False = timing-only probes)
    "pend": 5,           # S->U pipeline depth
    "ebufs": 8,          # e_sb buffer count
    "do_exp": True,      # False: replace exp with ACT copy (timing probe)
    "copies": "alt",     # psum->sbuf copy engine: alt | dve (keep ACT for exp)
    "xt_dma": "scalar",  # second-half xT load queue: scalar | sync | gpsimd
    "mask": "pre",       # crossing-tile mask: pre (add -1e4 to S, then exp) |
                         # post (exp raw S, then multiply e by 0/1 pattern)
}

_PROGRAM = None  # cached compiled Bass program


def _mult_patterns() -> np.ndarray:
    """[128, 512] 0/1 multiplicative mask bank (post-exp variant): cols 0:128
    tri01[j,i] = 0 if j <= i else 1; cols 128:512 all zero."""
    pat = np.zeros((P, 512), np.float32)
    j = np.arange(P)
    i = np.arange(P)
    pat[:, 0:P] = (j[:, None] > i[None, :]).astype(np.float32)
    return np.ascontiguousarray(pat)


def _adder_patterns() -> np.ndarray:
    """[128, 512] f32 mask-adder pattern bank.

    cols 0:128   "tri"  : tri[j,i] = -10000 if j <= i else 0 (the diagonal
                          128x128 block of the reverse-causal mask; identical
                          for every crossing tile)
    cols 128:512 "tail" : all -10000 (fully-masked region right of the
                          diagonal block)

    A crossing tile's adder over s[:, d:d+L] (d = diagonal start) is
    pat[:, 0:L].  Column i=1023 (local 511 of ic=1) stays raw in every tile
    -- jax softmax max-subtraction makes the fully-masked last row equal
    softmax of the raw scores -- so ic=1 adder spans stop at column 511.
    """
    pat = np.full((P, 512), -10000.0, np.float32)
    j = np.arange(P)
    i = np.arange(P)
    tri = np.where(j[:, None] <= i[None, :], -10000.0, 0.0).astype(np.float32)
    pat[:, 0:P] = tri
    return np.ascontiguousarray(pat)


def build_program(
    mm_dt: str = MM_DT, compile: bool = True, reps: int = 1, phase: str = "all"
):
    import concourse.bass as bass  # noqa: F401
    import concourse.tile as tile
    from concourse import bacc, mybir

    f32 = mybir.dt.float32
    use_f32r = mm_dt == "f32r"
    mdt = {
        "f32r": mybir.dt.float32r,
        "f32": mybir.dt.float32,
        "bf16": mybir.dt.bfloat16,
    }[mm_dt]
    Exp = mybir.ActivationFunctionType.Exp
    Copy = mybir.ActivationFunctionType.Copy
    ADD = mybir.AluOpType.add
    MUL = mybir.AluOpType.mult

    nc = bacc.Bacc(
        "TRN2",
        target_bir_lowering=False,
        debug=False,
        enable_asserts=False,
        num_devices=B,
    )

    xT_d = nc.dram_tensor("xT", [D, T], mdt, kind="ExternalInput")
    wq_d = nc.dram_tensor("wqT", [D, D], mdt, kind="ExternalInput")
    wk_d = nc.dram_tensor("wkT", [D, D], mdt, kind="ExternalInput")
    wv_d = nc.dram_tensor("wvT", [D, D], mdt, kind="ExternalInput")
    ad_d = nc.dram_tensor("adders", [P, 512], f32, kind="ExternalInput")
    mm_d = nc.dram_tensor("mmask", [P, 512], mdt, kind="ExternalInput")
    on_d = nc.dram_tensor("ones_t", [P, P], mdt, kind="ExternalInput")
    # out is stored TRANSPOSED ([D, T]); the host wrapper transposes back.
    out_d = nc.dram_tensor("out", [D, T], f32, kind="ExternalOutput")

    with tile.TileContext(nc) as tc:
        with (
            tc.tile_pool(name="sb", bufs=1) as sb,
            tc.tile_pool(name="ps", bufs=1, space="PSUM") as ps,
        ):
            KT_ORDER = (4, 0, 5, 1, 6, 2, 7, 3)

            def emit(rp=0):
                # rp = rep parity: resident tiles alternate SBUF slots across
                # benchmark reps so rep r+1's input DMAs overlap rep r's tail
                # instead of serializing on WAR hazards.
                # ---------------- resident loads ----------------
                # xT split across the SP and ACT DMA queues; weights on
                # Pool/SP; adder blocks trickle in on Pool in first-use
                # order.  kt loops consume in KT_ORDER = arrival order.
                xT = [None] * NKT

                def load_xT(k, eng):
                    t = sb.tile([P, T], mdt, tag=f"xT{k}p{rp}", name=f"xT{k}")
                    eng.dma_start(t[:], xT_d.ap()[P * k : P * (k + 1), :])
                    xT[k] = t

                for k in (4, 0, 5, 1):
                    load_xT(k, nc.sync)
                xt_eng = {"scalar": nc.scalar, "sync": nc.sync,
                          "gpsimd": nc.gpsimd}[KOPT["xt_dma"]]
                for k in (6, 2, 7, 3):
                    load_xT(k, xt_eng)
                pat = sb.tile([P, 512], f32, tag=f"pat{rp}", name="pat")
                xt_eng.dma_start(pat[:], ad_d.ap()[:])
                mpat = None
                if KOPT["mask"] == "post":
                    mpat = sb.tile([P, 512], mdt, tag=f"mpat{rp}", name="mpat")
                    xt_eng.dma_start(mpat[:], mm_d.ap()[:])

                copy_flip = [0]

                def psum_to_sbuf(dst_ap, src_ap, eng=None):
                    if eng is None:
                        if KOPT["copies"] == "dve":
                            eng = nc.vector
                        else:
                            eng = nc.scalar if copy_flip[0] % 2 == 0 else nc.vector
                            copy_flip[0] += 1
                    if eng is nc.scalar:
                        eng.activation(dst_ap, src_ap, Copy)
                    else:
                        eng.tensor_copy(dst_ap, src_ap)


                W, QKV = {}, {}

                def load_weights(g):
                    dram = {"wq": wq_d, "wk": wk_d, "wv": wv_d}
                    if g == 0:
                        placement = {
                            "wq": [(kt, nc.gpsimd) for kt in KT_ORDER],
                            "wk": [(4, nc.sync), (0, nc.sync), (5, nc.sync),
                                   (1, nc.sync), (6, nc.gpsimd), (2, nc.gpsimd),
                                   (7, nc.gpsimd), (3, nc.gpsimd)],
                            "wv": [(4, nc.sync), (0, nc.sync), (5, nc.sync),
                                   (1, nc.sync), (6, nc.gpsimd), (2, nc.gpsimd),
                                   (7, nc.gpsimd), (3, nc.gpsimd)],
                        }
                        order = ("wq", "wk", "wv")
                    else:
                        placement = {
                            nm: [(kt, nc.sync) for kt in KT_ORDER]
                            for nm in ("wv", "wq", "wk")
                        }
                        order = ("wv", "wq", "wk")
                    lists = {}
                    for nm in order:
                        lst = [None] * NKT
                        for kt, eng in placement[nm]:
                            w = sb.tile(
                                [P, NI], mdt, tag=f"{nm}{kt}p{rp}", name=f"{nm}{kt}g{g}"
                            )
                            eng.dma_start(
                                w[:],
                                dram[nm].ap()[P * kt : P * (kt + 1), NI * g : NI * (g + 1)],
                            )
                            lst[kt] = w
                        lists[nm] = lst
                    W[g] = (lists["wq"], lists["wk"], lists["wv"])
                    QKV[g] = (
                        [
                            sb.tile([P, T], mdt, tag=f"qT{ot}p{rp}", name=f"qT{ot}g{g}")
                            for ot in range(GH)
                        ],
                        [
                            sb.tile([P, T], mdt, tag=f"kT{ot}p{rp}", name=f"kT{ot}g{g}")
                            for ot in range(GH)
                        ],
                        [
                            sb.tile([P, NI], mdt, tag=f"v{tt}p{rp}", bufs=2, name=f"v{tt}g{g}")
                            for tt in range(NKT)
                        ],
                    )

                # ---------------- projection generators ----------------
                def proj_qk_gen(g, ot, copy_eng=None):
                    wq_g, wk_g, _ = W[g]
                    qT_g, kT_g, _ = QKV[g]
                    for wlist, dst in ((wq_g, qT_g[ot]), (wk_g, kT_g[ot])):
                        for tci in range(2):
                            pp = ps.tile([P, NI], f32, tag="pp", bufs=2, name="pp")
                            for ki, kt in enumerate(KT_ORDER):
                                nc.tensor.matmul(
                                    pp[:],
                                    wlist[kt][:, P * ot : P * (ot + 1)],
                                    xT[kt][:, NI * tci : NI * (tci + 1)],
                                    start=(ki == 0),
                                    stop=(ki == NKT - 1),
                                )
                            psum_to_sbuf(
                                dst[:, NI * tci : NI * (tci + 1)], pp[:], copy_eng
                            )
                            yield

                def proj_v_gen(g):
                    _, _, wv_g = W[g]
                    _, _, v_g = QKV[g]
                    # production order = attention's consumption order (full
                    # j-tiles 4..7 are drained first in every (head, ic) unit)
                    for tt in (4, 5, 6, 7, 0, 1, 2, 3):
                        pp = ps.tile([P, NI], f32, tag="pp", bufs=2, name="pp")
                        for ki, kt in enumerate(KT_ORDER):
                            nc.tensor.matmul(
                                pp[:],
                                xT[kt][:, P * tt : P * (tt + 1)],
                                wv_g[kt][:],
                                start=(ki == 0),
                                stop=(ki == NKT - 1),
                            )
                        psum_to_sbuf(v_g[tt][:], pp[:])
                        yield

                def wload_gen(g):
                    load_weights(g)
                    return
                    yield  # noqa: unreachable - makes this a generator

                # ---------------- attention generator ----------------
                def attn_gen(g, ot):
                    h = GH * g + ot
                    # eacc chain engine: Pool won the full-kernel sweep at
                    # pend=5 (keeps DVE free for the adder/recip/mul chain).
                    if KOPT["acc_eng"] == "alt":
                        acc_eng = nc.vector if h % 2 == 0 else nc.gpsimd
                    else:
                        acc_eng = nc.vector if KOPT["acc_eng"] == "dve" else nc.gpsimd
                    last_unit = g == NG - 1 and ot == GH - 1
                    qT_g, kT_g, v_g = QKV[g]
                    qh, kh = qT_g[ot], kT_g[ot]

                    # HW rejects fp32r matmuls with tiny output free size
                    # (s3d3_mm_fp32r_restrictions); run those as plain fp32.
                    def smallmm(ap):
                        return ap.bitcast(f32) if use_f32r else ap

                    # last unit runs ic=1 first: ic=0 has no exception path,
                    # so the end-of-program dependency tail is shorter
                    ic_order = (1, 0) if last_unit else (0, 1)
                    for ic in ic_order:
                        # (jt, width, adder_spec) per tile.  width = i-extent
                        # actually computed (rest of the row is exactly 0 after
                        # exp of the -10000 adder, so it is skipped outright).
                        # adder_spec = (dst_lo, dst_hi, pat_lo, pat_hi) or None.
                        # Widths stay >=256: f32r matmuls drop to 1/4 rate
                        # below a 256-wide moving dim (bf16 is safe either way).
                        # Full-width tiles come FIRST: the leading U-matmul
                        # (start=True) must cover the whole PSUM bank, and the
                        # first two eacc entries must be full tiles.
                        if ic == 0:
                            tiles = [
                                (4, NI, None),
                                (5, NI, None),
                                (6, NI, None),
                                (7, NI, None),
                                # jt0: diag at [0,128) + masked tail [128,256)
                                (0, 256, (0, 256, 0, 256)),
                                # jt1..3: pure diagonal block
                                (1, 256, (128, 256, 0, 128)),
                                (2, 384, (256, 384, 0, 128)),
                                (3, NI, (384, 512, 0, 128)),
                            ]
                        else:
                            # column i=1023 (local 511) must stay raw for every
                            # jt, so these tiles cannot be narrowed and every
                            # adder span stops at column 511.
                            tiles = [
                                (4, NI, (0, 511, 0, 511)),
                                (5, NI, (128, 511, 0, 383)),
                                (6, NI, (256, 511, 0, 255)),
                                (7, NI, (384, 511, 0, 127)),
                            ]
                        nj = len(tiles)

                        u_ps = ps.tile([P, NI], f32, tag="u", bufs=2, name="u_ps")
                        c_ps = ps.tile([P, NI], f32, tag="c", bufs=1, name="c_ps")

                        col_ps = colE = None
                        if ic == 1:
                            # raw scores for column i=1023, rows j in [0,512)
                            col_ps = ps.tile([P, 8], f32, tag="col", bufs=1, name="col_ps")
                            for jc in range(4):
                                nc.tensor.matmul(
                                    col_ps[:, jc : jc + 1],
                                    smallmm(kh[:, P * jc : P * (jc + 1)]),
                                    smallmm(qh[:, T - 1 : T]),
                                    start=True,
                                    stop=True,
                                )
                            colE = sb.tile([P, 8], mdt, tag="colE", bufs=2, name="colE")
                            nc.scalar.activation(colE[:, 0:4], col_ps[:, 0:4], Exp)

                        pend = []
                        eacc = [None]

                        def drain_one():
                            idx, jt, w, e_sb = pend.pop(0)
                            first, last = idx == 0, idx == nj - 1
                            nc.tensor.matmul(
                                u_ps[:, 0:w],
                                v_g[jt][:, P * ot : P * (ot + 1)],
                                e_sb[:, 0:w],
                                start=first,
                                stop=last,
                            )
                            # colsum via elementwise tile accumulation on
                            # acc_eng (SBUF-only chain); one ones-matmul at the
                            # end reduces partitions.
                            if idx == 0:
                                eacc[0] = e_sb
                            elif idx == 1:
                                acc = sb.tile(
                                    [P, NI], mdt, tag="eacc", bufs=2, name="eacc"
                                )
                                acc_eng.tensor_tensor(
                                    acc[:], eacc[0][:], e_sb[:], ADD
                                )
                                eacc[0] = acc
                            else:
                                acc_eng.tensor_tensor(
                                    eacc[0][:, 0:w], eacc[0][:, 0:w], e_sb[:, 0:w], ADD
                                )

                        for idx, (jt, w, aspec) in enumerate(tiles):
                            # last unit has no proj filler: borrow the idle pp
                            # psum banks to deepen the S pipeline
                            stag = "pp" if (last_unit and idx % 2 == 1) else "s"
                            s_ps = ps.tile([P, NI], f32, tag=stag, bufs=2, name="s_ps")
                            nc.tensor.matmul(
                                s_ps[:, 0:w],
                                kh[:, P * jt : P * (jt + 1)],
                                qh[:, NI * ic : NI * ic + w],
                                start=True,
                                stop=True,
                            )
                            post = KOPT["mask"] == "post"
                            if aspec is not None and KOPT["adders"] and not post:
                                dlo, dhi, plo, phi = aspec
                                nc.vector.tensor_tensor(
                                    s_ps[:, dlo:dhi],
                                    s_ps[:, dlo:dhi],
                                    pat[:, plo:phi],
                                    ADD,
                                )
                            e_sb = sb.tile(
                                [P, NI], mdt, tag="e", bufs=KOPT["ebufs"], name="e_sb"
                            )
                            nc.scalar.activation(
                                e_sb[:, 0:w], s_ps[:, 0:w],
                                Exp if KOPT["do_exp"] else Copy,
                            )
                            if aspec is not None and KOPT["adders"] and post:
                                dlo, dhi, plo, phi = aspec
                                nc.vector.tensor_tensor(
                                    e_sb[:, dlo:dhi],
                                    e_sb[:, dlo:dhi],
                                    mpat[:, plo:phi],
                                    MUL,
                                )
                            pend.append((idx, jt, w, e_sb))
                            while len(pend) > KOPT["pend"]:
                                drain_one()
                            yield
                        while pend:
                            drain_one()
                        nc.tensor.matmul(
                            c_ps[:], ones[:], eacc[0][:], start=True, stop=True
                        )

                        if ic == 1:
                            # fold the j<512 contributions of column i=1023 in
                            for jc in range(4):
                                nc.tensor.matmul(
                                    col_ps[:, 4:5],
                                    smallmm(v_g[jc][:, P * ot : P * (ot + 1)]),
                                    smallmm(colE[:, jc : jc + 1]),
                                    start=(jc == 0),
                                    stop=(jc == 3),
                                )
                            for jc in range(4):
                                nc.tensor.matmul(
                                    col_ps[:, 5:6],
                                    smallmm(ones[:]),
                                    smallmm(colE[:, jc : jc + 1]),
                                    start=(jc == 0),
                                    stop=(jc == 3),
                                )
                            colsb = sb.tile([P, 2], f32, tag="colsb", bufs=2, name="colsb")
                            nc.scalar.activation(colsb[:], col_ps[:, 4:6], Copy)
                            nc.vector.tensor_tensor(
                                u_ps[:, NI - 1 : NI], u_ps[:, NI - 1 : NI], colsb[:, 0:1], ADD
                            )
                            nc.vector.tensor_tensor(
                                c_ps[:, NI - 1 : NI], c_ps[:, NI - 1 : NI], colsb[:, 1:2], ADD
                            )

                        recip = sb.tile([P, NI], f32, tag="recip", bufs=2, name="recip")
                        o_sb = sb.tile([P, NI], f32, tag="o", bufs=3, name="o_sb")
                        if last_unit and ic == 0:
                            # final epilogue is fully exposed: halve the DVE
                            # chain so the first out-DMA overlaps the second
                            hn = NI // 2
                            for hf in range(2):
                                sl = slice(hn * hf, hn * (hf + 1))
                                nc.vector.reciprocal(recip[:, sl], c_ps[:, sl])
                                nc.vector.tensor_tensor(
                                    o_sb[:, sl], u_ps[:, sl], recip[:, sl], MUL
                                )
                                nc.sync.dma_start(
                                    out_d.ap()[
                                        P * h : P * (h + 1),
                                        NI * ic + hn * hf : NI * ic + hn * (hf + 1),
                                    ],
                                    o_sb[:, sl],
                                )
                        else:
                            nc.vector.reciprocal(recip[:], c_ps[:])
                            nc.vector.tensor_tensor(o_sb[:], u_ps[:], recip[:], MUL)
                            nc.sync.dma_start(
                                out_d.ap()[P * h : P * (h + 1), NI * ic : NI * (ic + 1)],
                                o_sb[:],
                            )
                        yield

                # ---------------- schedule ----------------
                if phase == "proj":
                    # timing-isolation variant: projections only
                    load_weights(0)
                    for g in range(NG):
                        if g == 1:
                            load_weights(1)
                        for ot in range(GH):
                            for _ in proj_qk_gen(g, ot):
                                pass
                        for _ in proj_v_gen(g):
                            pass
                    for ot in range(GH):
                        qT_g, _, _ = QKV[1]
                        nc.gpsimd.dma_start(
                            out_d.ap()[P * ot : P * (ot + 1), :], qT_g[ot][:]
                        )
                    return
                if phase == "attn":
                    # timing-isolation variant: attention only, q/k/v faked
                    # by direct DRAM loads (W tensors reinterpreted)
                    ones = sb.tile([P, P], mdt, tag=f"ones{rp}", name="ones")
                    nc.gpsimd.dma_start(ones[:], on_d.ap()[:])
                    for g in range(NG):
                        qts = [
                            sb.tile([P, T], mdt, tag=f"qT{ot}p{rp}", name=f"qT{ot}g{g}")
                            for ot in range(GH)
                        ]
                        kts = [
                            sb.tile([P, T], mdt, tag=f"kT{ot}p{rp}", name=f"kT{ot}g{g}")
                            for ot in range(GH)
                        ]
                        vts = [
                            sb.tile([P, NI], mdt, tag=f"v{tt}p{rp}", bufs=2,
                                    name=f"v{tt}g{g}")
                            for tt in range(NKT)
                        ]
                        for ot in range(GH):
                            nc.sync.dma_start(
                                qts[ot][:], wq_d.ap()[P * ot : P * (ot + 1), :]
                            )
                            nc.scalar.dma_start(
                                kts[ot][:], wk_d.ap()[P * ot : P * (ot + 1), :]
                            )
                        for tt in range(NKT):
                            nc.gpsimd.dma_start(
                                vts[tt][:],
                                wv_d.ap()[P * tt : P * (tt + 1), 0:NI],
                            )
                        QKV[g] = (qts, kts, vts)
                    for g in range(NG):
                        for ot in range(GH):
                            for _ in attn_gen(g, ot):
                                pass
                    return
                load_weights(0)
                ones = sb.tile([P, P], mdt, tag=f"ones{rp}", name="ones")
                nc.gpsimd.dma_start(ones[:], on_d.ap()[:])
                for _ in proj_qk_gen(0, 0, copy_eng=nc.vector):
                    pass

                # Filler generators are window-scoped: proj work for (g1, ot)
                # may only be emitted strictly after attn(g0, ot) has finished
                # emitting (WAR hazards on the single-buffered qT/kT/w tiles
                # would otherwise deadlock the in-order engine queues).
                windows = {
                    (0, 0): [proj_v_gen(0), proj_qk_gen(0, 1)],
                    (0, 1): [proj_qk_gen(0, 2)],
                    (0, 2): [proj_qk_gen(0, 3), wload_gen(1), proj_qk_gen(1, 0)],
                    (0, 3): [proj_v_gen(1)],
                    (1, 0): [proj_qk_gen(1, 1)],
                    (1, 1): [proj_qk_gen(1, 2)],
                    (1, 2): [proj_qk_gen(1, 3)],
                }

                for g in range(NG):
                    for ot in range(GH):
                        filler = deque(windows.get((g, ot), []))

                        def pump(n):
                            while n > 0 and filler:
                                try:
                                    next(filler[0])
                                    n -= 1
                                except StopIteration:
                                    filler.popleft()

                        for _ in attn_gen(g, ot):
                            pump(1)
                        pump(10**9)  # drain before the next unit starts

            for _rep in range(reps):
                emit(_rep % 2)

    if compile:
        nc.compile()
    return nc


def _get_program():
    global _PROGRAM
    if _PROGRAM is None:
        _PROGRAM = build_program()
    return _PROGRAM


def make_in_maps(x, Wq, Wk, Wv):
    if MM_DT == "bf16":
        import ml_dtypes

        op_dt = ml_dtypes.bfloat16
    else:
        op_dt = np.float32
    scale = 1.0 / np.sqrt(np.float32(DH))
    wqT = np.ascontiguousarray((np.asarray(Wq, np.float32).T * scale).astype(op_dt))
    wkT = np.ascontiguousarray(np.asarray(Wk, np.float32).T.astype(op_dt))
    wvT = np.ascontiguousarray(np.asarray(Wv, np.float32).T.astype(op_dt))
    adders = _adder_patterns()
    mmask = _mult_patterns().astype(op_dt)
    ones = np.ones((P, P), op_dt)
    x = np.asarray(x, np.float32)
    in_maps = []
    for b in range(B):
        in_maps.append(
            {
                "xT": np.ascontiguousarray(x[b].T.astype(op_dt)),
                "wqT": wqT,
                "wkT": wkT,
                "wvT": wvT,
                "adders": adders,
                "mmask": mmask,
                "ones_t": ones,
            }
        )
    return in_maps


def kernel(x, mask, Wq, Wk, Wv, _trace=False):
    from concourse.bass_utils import run_bass_kernel_spmd

    nc = _get_program()
    in_maps = make_in_maps(x, Wq, Wk, Wv)
    res = run_bass_kernel_spmd(nc, in_maps, core_ids=list(range(B)), trace=_trace)
    out = np.stack([res.results[b]["out"] for b in range(B)], axis=0)
    out = np.swapaxes(out, 1, 2)  # device stores out.T
    out = out * np.asarray(mask, np.float32)[:, :, None]
    out = np.ascontiguousarray(out, np.float32)
    if _trace:
        kernel.last_results = res
    return out



# revision 29
# speedup vs baseline: 1.2538x; 1.2538x over previous
"""Trainium2 Bass kernel for causal ("FORWARD" direction) multi-head attention.

Reference computation (per batch b, n_heads=8, d=128):
  Q = x @ Wq.T ; K = x @ Wk.T ; V = x @ Wv.T          (nn.Linear, no bias)
  scores[h,i,j] = (Qh[i] . Kh[j]) / sqrt(d)
  scores += -10000 where j <= i   (keeps strict upper triangle j > i)
  attn = softmax(scores, axis=j) ; out = attn @ Vh ; concat heads
  Row i=1023 is fully masked; jax softmax's max-subtraction makes it equal
  softmax of the *raw* scores, so the kernel keeps column i=1023 unmasked.

Sharding: data-parallel over batch B=8 -> 8 cores, no collectives.

Device layout (per core, everything transposed so the softmax reduction is a
matmul-friendly partition-dim reduction):
  xT[k,t]       : x.T                                  [1024,1024]
  qT/kT[o,t]    : per head-group of 4 heads            via Wq.T/Wk.T as lhsT
  v[t,o]        : natural V                            via xT as lhsT
  S_T[j,i]      = kT_tile.T @ qT  (contraction over d=128, single tile)
  expS          = exp(S_T + adder)   (adder patterns precomputed on host)
  U_T[dd,i]     = sum_j V[j,dd] expS[j,i]   (matmul accum over j tiles)
  colsum[*,i]   = ones.T @ expS             (partition-broadcast row of sums)
  out_T         = U_T * reciprocal(colsum) -> DRAM (host transposes back)

Scheduling: projections for the NEXT head-group are emitted interleaved with
attention of the current group, so PE fills the gaps where it would otherwise
wait on Activation (exp) results.
"""

import os
import sys
from collections import deque

import numpy as np

if "/opt/trn_rl_repo" not in sys.path:
    sys.path.insert(0, "/opt/trn_rl_repo")

B, T, D, H, DH = 8, 1024, 1024, 8, 128
P = 128          # partition tile
NI = 512         # i-chunk (moving free size)
NG, GH = 2, 4    # head groups x heads per group
NKT = T // P     # 8 contraction tiles
MM_DT = os.environ.get("KERNEL_MM_DT", "bf16")  # f32r | f32 | bf16

# experiment knobs (production defaults; probes override via module attr)
KOPT = {
    "acc_eng": "alt",   # eacc chain engine: dve | pool | alt(ernate by unit)
    "adders": True,      # apply mask adders (False = timing-only probes)
    "pend": 4,           # S->U pipeline depth
    "ebufs": 8,          # e_sb buffer count
    "do_exp": True,      # False: replace exp with ACT copy (timing probe)
    "copies": "alt",     # psum->sbuf copy engine: alt | dve (keep ACT for exp)
    "xt_dma": "scalar",  # second-half xT load queue: scalar | sync | gpsimd
    "mask": "pre",       # crossing-tile mask: pre (add -1e4 to S, then exp) |
                         # post (exp raw S, then multiply e by 0/1 pattern)
}

_PROGRAM = None  # cached compiled Bass program


def _mult_patterns() -> np.ndarray:
    """[128, 512] 0/1 multiplicative mask bank (post-exp variant): cols 0:128
    tri01[j,i] = 0 if j <= i else 1; cols 128:512 all zero."""
    pat = np.zeros((P, 512), np.float32)
    j = np.arange(P)
    i = np.arange(P)
    pat[:, 0:P] = (j[:, None] > i[None, :]).astype(np.float32)
    return np.ascontiguousarray(pat)


def _adder_patterns() -> np.ndarray:
    """[128, 512] f32 mask-adder pattern bank.

    cols 0:128   "tri"  : tri[j,i] = -10000 if j <= i else 0 (the diagonal
                          128x128 block of the reverse-causal mask; identical
                          for every crossing tile)
    cols 128:512 "tail" : all -10000 (fully-masked region right of the
                          diagonal block)

    A crossing tile's adder over s[:, d:d+L] (d = diagonal start) is
    pat[:, 0:L].  Column i=1023 (local 511 of ic=1) stays raw in every tile
    -- jax softmax max-subtraction makes the fully-masked last row equal
    softmax of the raw scores -- so ic=1 adder spans stop at column 511.
    """
    pat = np.full((P, 512), -10000.0, np.float32)
    j = np.arange(P)
    i = np.arange(P)
    tri = np.where(j[:, None] <= i[None, :], -10000.0, 0.0).astype(np.float32)
    pat[:, 0:P] = tri
    return np.ascontiguousarray(pat)


def build_program(
    mm_dt: str = MM_DT, compile: bool = True, reps: int = 1, phase: str = "all"
):
    import concourse.bass as bass  # noqa: F401
    import concourse.tile as tile
    from concourse import bacc, mybir

    f32 = mybir.dt.float32
    use_f32r = mm_dt == "f32r"
    mdt = {
        "f32r": mybir.dt.float32r,
        "f32": mybir.dt.float32,
        "bf16": mybir.dt.bfloat16,
    }[mm_dt]
    Exp = mybir.ActivationFunctionType.Exp
    Copy = mybir.ActivationFunctionType.Copy
    ADD = mybir.AluOpType.add
    MUL = mybir.AluOpType.mult

    nc = bacc.Bacc(
        "TRN2",
        target_bir_lowering=False,
        debug=False,
        enable_asserts=False,
        num_devices=B,
    )

    xT_d = nc.dram_tensor("xT", [D, T], mdt, kind="ExternalInput")
    wq_d = nc.dram_tensor("wqT", [D, D], mdt, kind="ExternalInput")
    wk_d = nc.dram_tensor("wkT", [D, D], mdt, kind="ExternalInput")
    wv_d = nc.dram_tensor("wvT", [D, D], mdt, kind="ExternalInput")
    ad_d = nc.dram_tensor("adders", [P, 512], f32, kind="ExternalInput")
    mm_d = nc.dram_tensor("mmask", [P, 512], mdt, kind="ExternalInput")
    on_d = nc.dram_tensor("ones_t", [P, P], mdt, kind="ExternalInput")
    # out is stored TRANSPOSED ([D, T]); the host wrapper transposes back.
    out_d = nc.dram_tensor("out", [D, T], f32, kind="ExternalOutput")

    with tile.TileContext(nc) as tc:
        with (
            tc.tile_pool(name="sb", bufs=1) as sb,
            tc.tile_pool(name="ps", bufs=1, space="PSUM") as ps,
        ):
            KT_ORDER = (4, 0, 5, 1, 6, 2, 7, 3)

            def emit(rp=0):
                # rp = rep parity: resident tiles alternate SBUF slots across
                # benchmark reps so rep r+1's input DMAs overlap rep r's tail
                # instead of serializing on WAR hazards.
                # ---------------- resident loads ----------------
                # xT split across the SP and ACT DMA queues; weights on
                # Pool/SP; adder blocks trickle in on Pool in first-use
                # order.  kt loops consume in KT_ORDER = arrival order.
                xT = [None] * NKT

                def load_xT(k, eng):
                    t = sb.tile([P, T], mdt, tag=f"xT{k}p{rp}", name=f"xT{k}")
                    eng.dma_start(t[:], xT_d.ap()[P * k : P * (k + 1), :])
                    xT[k] = t

                for k in (4, 0, 5, 1):
                    load_xT(k, nc.sync)
                xt_eng = {"scalar": nc.scalar, "sync": nc.sync,
                          "gpsimd": nc.gpsimd}[KOPT["xt_dma"]]
                for k in (6, 2, 7, 3):
                    load_xT(k, xt_eng)
                pat = sb.tile([P, 512], f32, tag=f"pat{rp}", name="pat")
                xt_eng.dma_start(pat[:], ad_d.ap()[:])
                mpat = None
                if KOPT["mask"] == "post":
                    mpat = sb.tile([P, 512], mdt, tag=f"mpat{rp}", name="mpat")
                    xt_eng.dma_start(mpat[:], mm_d.ap()[:])

                copy_flip = [0]

                def psum_to_sbuf(dst_ap, src_ap, eng=None):
                    if eng is None:
                        if KOPT["copies"] == "dve":
                            eng = nc.vector
                        else:
                            eng = nc.scalar if copy_flip[0] % 2 == 0 else nc.vector
                            copy_flip[0] += 1
                    if eng is nc.scalar:
                        eng.activation(dst_ap, src_ap, Copy)
                    else:
                        eng.tensor_copy(dst_ap, src_ap)


                W, QKV = {}, {}

                def load_weights(g):
                    dram = {"wq": wq_d, "wk": wk_d, "wv": wv_d}
                    if g == 0:
                        placement = {
                            "wq": [(kt, nc.gpsimd) for kt in KT_ORDER],
                            "wk": [(4, nc.sync), (0, nc.sync), (5, nc.sync),
                                   (1, nc.sync), (6, nc.gpsimd), (2, nc.gpsimd),
                                   (7, nc.gpsimd), (3, nc.gpsimd)],
                            "wv": [(4, nc.sync), (0, nc.sync), (5, nc.sync),
                                   (1, nc.sync), (6, nc.gpsimd), (2, nc.gpsimd),
                                   (7, nc.gpsimd), (3, nc.gpsimd)],
                        }
                        order = ("wq", "wk", "wv")
                    else:
                        placement = {
                            nm: [(kt, nc.sync) for kt in KT_ORDER]
                            for nm in ("wv", "wq", "wk")
                        }
                        order = ("wv", "wq", "wk")
                    lists = {}
                    for nm in order:
                        lst = [None] * NKT
                        for kt, eng in placement[nm]:
                            w = sb.tile(
                                [P, NI], mdt, tag=f"{nm}{kt}p{rp}", name=f"{nm}{kt}g{g}"
                            )
                            eng.dma_start(
                                w[:],
                                dram[nm].ap()[P * kt : P * (kt + 1), NI * g : NI * (g + 1)],
                            )
                            lst[kt] = w
                        lists[nm] = lst
                    W[g] = (lists["wq"], lists["wk"], lists["wv"])
                    QKV[g] = (
                        [
                            sb.tile([P, T], mdt, tag=f"qT{ot}p{rp}", name=f"qT{ot}g{g}")
                            for ot in range(GH)
                        ],
                        [
                            sb.tile([P, T], mdt, tag=f"kT{ot}p{rp}", name=f"kT{ot}g{g}")
                            for ot in range(GH)
                        ],
                        [
                            sb.tile([P, NI], mdt, tag=f"v{tt}p{rp}", bufs=2, name=f"v{tt}g{g}")
                            for tt in range(NKT)
                        ],
                    )

                # ---------------- projection generators ----------------
                def proj_qk_gen(g, ot, copy_eng=None):
                    wq_g, wk_g, _ = W[g]
                    qT_g, kT_g, _ = QKV[g]
                    for wlist, dst in ((wq_g, qT_g[ot]), (wk_g, kT_g[ot])):
                        for tci in range(2):
                            pp = ps.tile([P, NI], f32, tag="pp", bufs=2, name="pp")
                            for ki, kt in enumerate(KT_ORDER):
                                nc.tensor.matmul(
                                    pp[:],
                                    wlist[kt][:, P * ot : P * (ot + 1)],
                                    xT[kt][:, NI * tci : NI * (tci + 1)],
                                    start=(ki == 0),
                                    stop=(ki == NKT - 1),
                                )
                            psum_to_sbuf(
                                dst[:, NI * tci : NI * (tci + 1)], pp[:], copy_eng
                            )
                            yield

                def proj_v_gen(g):
                    _, _, wv_g = W[g]
                    _, _, v_g = QKV[g]
                    # production order = attention's consumption order (full
                    # j-tiles 4..7 are drained first in every (head, ic) unit)
                    for tt in (4, 5, 6, 7, 0, 1, 2, 3):
                        pp = ps.tile([P, NI], f32, tag="pp", bufs=2, name="pp")
                        for ki, kt in enumerate(KT_ORDER):
                            nc.tensor.matmul(
                                pp[:],
                                xT[kt][:, P * tt : P * (tt + 1)],
                                wv_g[kt][:],
                                start=(ki == 0),
                                stop=(ki == NKT - 1),
                            )
                        psum_to_sbuf(v_g[tt][:], pp[:])
                        yield

                def wload_gen(g):
                    load_weights(g)
                    return
                    yield  # noqa: unreachable - makes this a generator

                # ---------------- attention generator ----------------
                def attn_gen(g, ot):
                    h = GH * g + ot
                    # eacc chain engine: Pool won the full-kernel sweep at
                    # pend=5 (keeps DVE free for the adder/recip/mul chain).
                    if KOPT["acc_eng"] == "alt":
                        acc_eng = nc.vector if h % 2 == 0 else nc.gpsimd
                    else:
                        acc_eng = nc.vector if KOPT["acc_eng"] == "dve" else nc.gpsimd
                    last_unit = g == NG - 1 and ot == GH - 1
                    qT_g, kT_g, v_g = QKV[g]
                    qh, kh = qT_g[ot], kT_g[ot]

                    # HW rejects fp32r matmuls with tiny output free size
                    # (s3d3_mm_fp32r_restrictions); run those as plain fp32.
                    def smallmm(ap):
                        return ap.bitcast(f32) if use_f32r else ap

                    # last unit runs ic=1 first: ic=0 has no exception path,
                    # so the end-of-program dependency tail is shorter
                    ic_order = (1, 0) if last_unit else (0, 1)
                    for ic in ic_order:
                        # (jt, width, adder_spec) per tile.  width = i-extent
                        # actually computed (rest of the row is exactly 0 after
                        # exp of the -10000 adder, so it is skipped outright).
                        # adder_spec = (dst_lo, dst_hi, pat_lo, pat_hi) or None.
                        # Widths stay >=256: f32r matmuls drop to 1/4 rate
                        # below a 256-wide moving dim (bf16 is safe either way).
                        # Full-width tiles come FIRST: the leading U-matmul
                        # (start=True) must cover the whole PSUM bank, and the
                        # first two eacc entries must be full tiles.
                        if ic == 0:
                            tiles = [
                                (4, NI, None),
                                (5, NI, None),
                                (6, NI, None),
                                (7, NI, None),
                                # jt0: diag at [0,128) + masked tail [128,256)
                                (0, 256, (0, 256, 0, 256)),
                                # jt1..3: pure diagonal block
                                (1, 256, (128, 256, 0, 128)),
                                (2, 384, (256, 384, 0, 128)),
                                (3, NI, (384, 512, 0, 128)),
                            ]
                        else:
                            # column i=1023 (local 511) must stay raw for every
                            # jt, so these tiles cannot be narrowed and every
                            # adder span stops at column 511.
                            tiles = [
                                (4, NI, (0, 511, 0, 511)),
                                (5, NI, (128, 511, 0, 383)),
                                (6, NI, (256, 511, 0, 255)),
                                (7, NI, (384, 511, 0, 127)),
                            ]
                        nj = len(tiles)

                        u_ps = ps.tile([P, NI], f32, tag="u", bufs=2, name="u_ps")
                        c_ps = ps.tile([P, NI], f32, tag="c", bufs=1, name="c_ps")

                        col_ps = colE = None
                        if ic == 1:
                            # raw scores for column i=1023, rows j in [0,512)
                            col_ps = ps.tile([P, 8], f32, tag="col", bufs=1, name="col_ps")
                            for jc in range(4):
                                nc.tensor.matmul(
                                    col_ps[:, jc : jc + 1],
                                    smallmm(kh[:, P * jc : P * (jc + 1)]),
                                    smallmm(qh[:, T - 1 : T]),
                                    start=True,
                                    stop=True,
                                )
                            colE = sb.tile([P, 8], mdt, tag="colE", bufs=2, name="colE")
                            nc.scalar.activation(colE[:, 0:4], col_ps[:, 0:4], Exp)

                        pend = []
                        eacc = [None]

                        def drain_one():
                            idx, jt, w, e_sb = pend.pop(0)
                            first, last = idx == 0, idx == nj - 1
                            nc.tensor.matmul(
                                u_ps[:, 0:w],
                                v_g[jt][:, P * ot : P * (ot + 1)],
                                e_sb[:, 0:w],
                                start=first,
                                stop=last,
                            )
                            # colsum via elementwise tile accumulation on
                            # acc_eng (SBUF-only chain); one ones-matmul at the
                            # end reduces partitions.
                            if idx == 0:
                                eacc[0] = e_sb
                            elif idx == 1:
                                acc = sb.tile(
                                    [P, NI], mdt, tag="eacc", bufs=2, name="eacc"
                                )
                                acc_eng.tensor_tensor(
                                    acc[:], eacc[0][:], e_sb[:], ADD
                                )
                                eacc[0] = acc
                            else:
                                acc_eng.tensor_tensor(
                                    eacc[0][:, 0:w], eacc[0][:, 0:w], e_sb[:, 0:w], ADD
                                )

                        for idx, (jt, w, aspec) in enumerate(tiles):
                            # last unit has no proj filler: borrow the idle pp
                            # psum banks to deepen the S pipeline
                            stag = "pp" if (last_unit and idx % 2 == 1) else "s"
                            s_ps = ps.tile([P, NI], f32, tag=stag, bufs=2, name="s_ps")
                            nc.tensor.matmul(
                                s_ps[:, 0:w],
                                kh[:, P * jt : P * (jt + 1)],
                                qh[:, NI * ic : NI * ic + w],
                                start=True,
                                stop=True,
                            )
                            post = KOPT["mask"] == "post"
                            if aspec is not None and KOPT["adders"] and not post:
                                dlo, dhi, plo, phi = aspec
                                nc.vector.tensor_tensor(
                                    s_ps[:, dlo:dhi],
                                    s_ps[:, dlo:dhi],
                                    pat[:, plo:phi],
                                    ADD,
                                )
                            e_sb = sb.tile(
                                [P, NI], mdt, tag="e", bufs=KOPT["ebufs"], name="e_sb"
                            )
                            nc.scalar.activation(
                                e_sb[:, 0:w], s_ps[:, 0:w],
                                Exp if KOPT["do_exp"] else Copy,
                            )
                            if aspec is not None and KOPT["adders"] and post:
                                dlo, dhi, plo, phi = aspec
                                nc.vector.tensor_tensor(
                                    e_sb[:, dlo:dhi],
                                    e_sb[:, dlo:dhi],
                                    mpat[:, plo:phi],
                                    MUL,
                                )
                            pend.append((idx, jt, w, e_sb))
                            while len(pend) > KOPT["pend"]:
                                drain_one()
                            yield
                        while pend:
                            drain_one()
                        nc.tensor.matmul(
                            c_ps[:], ones[:], eacc[0][:], start=True, stop=True
                        )

                        if ic == 1:
                            # fold the j<512 contributions of column i=1023 in
                            for jc in range(4):
                                nc.tensor.matmul(
                                    col_ps[:, 4:5],
                                    smallmm(v_g[jc][:, P * ot : P * (ot + 1)]),
                                    smallmm(colE[:, jc : jc + 1]),
                                    start=(jc == 0),
                                    stop=(jc == 3),
                                )
                            for jc in range(4):
                                nc.tensor.matmul(
                                    col_ps[:, 5:6],
                                    smallmm(ones[:]),
                                    smallmm(colE[:, jc : jc + 1]),
                                    start=(jc == 0),
                                    stop=(jc == 3),
                                )
                            colsb = sb.tile([P, 2], f32, tag="colsb", bufs=2, name="colsb")
                            nc.scalar.activation(colsb[:], col_ps[:, 4:6], Copy)
                            nc.vector.tensor_tensor(
                                u_ps[:, NI - 1 : NI], u_ps[:, NI - 1 : NI], colsb[:, 0:1], ADD
                            )
                            nc.vector.tensor_tensor(
                                c_ps[:, NI - 1 : NI], c_ps[:, NI - 1 : NI], colsb[:, 1:2], ADD
                            )

                        recip = sb.tile([P, NI], f32, tag="recip", bufs=2, name="recip")
                        o_sb = sb.tile([P, NI], f32, tag="o", bufs=3, name="o_sb")
                        if last_unit and ic == 0:
                            # final epilogue is fully exposed: halve the DVE
                            # chain so the first out-DMA overlaps the second
                            hn = NI // 2
                            for hf in range(2):
                                sl = slice(hn * hf, hn * (hf + 1))
                                nc.vector.reciprocal(recip[:, sl], c_ps[:, sl])
                                nc.vector.tensor_tensor(
                                    o_sb[:, sl], u_ps[:, sl], recip[:, sl], MUL
                                )
                                nc.sync.dma_start(
                                    out_d.ap()[
                                        P * h : P * (h + 1),
                                        NI * ic + hn * hf : NI * ic + hn * (hf + 1),
                                    ],
                                    o_sb[:, sl],
                                )
                        else:
                            nc.vector.reciprocal(recip[:], c_ps[:])
                            nc.vector.tensor_tensor(o_sb[:], u_ps[:], recip[:], MUL)
                            nc.sync.dma_start(
                                out_d.ap()[P * h : P * (h + 1), NI * ic : NI * (ic + 1)],
                                o_sb[:],
                            )
                        yield

                # ---------------- schedule ----------------
                if phase == "proj":
                    # timing-isolation variant: projections only
                    load_weights(0)
                    for g in range(NG):
                        if g == 1:
                            load_weights(1)
                        for ot in range(GH):
                            for _ in proj_qk_gen(g, ot):
                                pass
                        for _ in proj_v_gen(g):
                            pass
                    for ot in range(GH):
                        qT_g, _, _ = QKV[1]
                        nc.gpsimd.dma_start(
                            out_d.ap()[P * ot : P * (ot + 1), :], qT_g[ot][:]
                        )
                    return
                if phase == "attn":
                    # timing-isolation variant: attention only, q/k/v faked
                    # by direct DRAM loads (W tensors reinterpreted)
                    ones = sb.tile([P, P], mdt, tag=f"ones{rp}", name="ones")
                    nc.gpsimd.dma_start(ones[:], on_d.ap()[:])
                    for g in range(NG):
                        qts = [
                            sb.tile([P, T], mdt, tag=f"qT{ot}p{rp}", name=f"qT{ot}g{g}")
                            for ot in range(GH)
                        ]
                        kts = [
                            sb.tile([P, T], mdt, tag=f"kT{ot}p{rp}", name=f"kT{ot}g{g}")
                            for ot in range(GH)
                        ]
                        vts = [
                            sb.tile([P, NI], mdt, tag=f"v{tt}p{rp}", bufs=2,
                                    name=f"v{tt}g{g}")
                            for tt in range(NKT)
                        ]
                        for ot in range(GH):
                            nc.sync.dma_start(
                                qts[ot][:], wq_d.ap()[P * ot : P * (ot + 1), :]
                            )
                            nc.scalar.dma_start(
                                kts[ot][:], wk_d.ap()[P * ot : P * (ot + 1), :]
                            )
                        for tt in range(NKT):
                            nc.gpsimd.dma_start(
                                vts[tt][:],
                                wv_d.ap()[P * tt : P * (tt + 1), 0:NI],
                            )
                        QKV[g] = (qts, kts, vts)
                    for g in range(NG):
                        for ot in range(GH):
                            for _ in attn_gen(g, ot):
                                pass
                    return
                load_weights(0)
                ones = sb.tile([P, P], mdt, tag=f"ones{rp}", name="ones")
                nc.gpsimd.dma_start(ones[:], on_d.ap()[:])
                for _ in proj_qk_gen(0, 0, copy_eng=nc.vector):
                    pass

                # Filler generators are window-scoped: proj work for (g1, ot)
                # may only be emitted strictly after attn(g0, ot) has finished
                # emitting (WAR hazards on the single-buffered qT/kT/w tiles
                # would otherwise deadlock the in-order engine queues).
                windows = {
                    (0, 0): [proj_v_gen(0), proj_qk_gen(0, 1)],
                    (0, 1): [proj_qk_gen(0, 2)],
                    (0, 2): [proj_qk_gen(0, 3), wload_gen(1), proj_qk_gen(1, 0)],
                    (0, 3): [proj_v_gen(1)],
                    (1, 0): [proj_qk_gen(1, 1)],
                    (1, 1): [proj_qk_gen(1, 2)],
                    (1, 2): [proj_qk_gen(1, 3)],
                }

                for g in range(NG):
                    for ot in range(GH):
                        filler = deque(windows.get((g, ot), []))

                        def pump(n):
                            while n > 0 and filler:
                                try:
                                    next(filler[0])
                                    n -= 1
                                except StopIteration:
                                    filler.popleft()

                        for _ in attn_gen(g, ot):
                            pump(1)
                        pump(10**9)  # drain before the next unit starts

            for _rep in range(reps):
                emit(_rep % 2)

    if compile:
        nc.compile()
    return nc


def _get_program():
    global _PROGRAM
    if _PROGRAM is None:
        _PROGRAM = build_program()
    return _PROGRAM


def make_in_maps(x, Wq, Wk, Wv):
    if MM_DT == "bf16":
        import ml_dtypes

        op_dt = ml_dtypes.bfloat16
    else:
        op_dt = np.float32
    scale = 1.0 / np.sqrt(np.float32(DH))
    wqT = np.ascontiguousarray((np.asarray(Wq, np.float32).T * scale).astype(op_dt))
    wkT = np.ascontiguousarray(np.asarray(Wk, np.float32).T.astype(op_dt))
    wvT = np.ascontiguousarray(np.asarray(Wv, np.float32).T.astype(op_dt))
    adders = _adder_patterns()
    mmask = _mult_patterns().astype(op_dt)
    ones = np.ones((P, P), op_dt)
    x = np.asarray(x, np.float32)
    in_maps = []
    for b in range(B):
        in_maps.append(
            {
                "xT": np.ascontiguousarray(x[b].T.astype(op_dt)),
                "wqT": wqT,
                "wkT": wkT,
                "wvT": wvT,
                "adders": adders,
                "mmask": mmask,
                "ones_t": ones,
            }
        )
    return in_maps


def kernel(x, mask, Wq, Wk, Wv, _trace=False):
    from concourse.bass_utils import run_bass_kernel_spmd

    nc = _get_program()
    in_maps = make_in_maps(x, Wq, Wk, Wv)
    res = run_bass_kernel_spmd(nc, in_maps, core_ids=list(range(B)), trace=_trace)
    out = np.stack([res.results[b]["out"] for b in range(B)], axis=0)
    out = np.swapaxes(out, 1, 2)  # device stores out.T
    out = out * np.asarray(mask, np.float32)[:, :, None]
    out = np.ascontiguousarray(out, np.float32)
    if _trace:
        kernel.last_results = res
    return out

